# revision 1
# baseline (speedup 1.0000x reference)
"""Trainium2 Bass kernel for the DispaxD3 two-body dispersion energy.

Strategy (8 NeuronCores, SPMD):
  - Edges are sorted by their i-atom and sharded across cores at atom
    boundaries (each core owns a contiguous atom range and all edges whose
    i-atom falls in it).  Per-core edge slots are laid out in degree-bucketed
    padded runs [128 partitions, n_atom_cols, L], so the per-atom segment sum
    (coordination number) and the per-atom broadcasts are regular strided
    vector ops.
  - Launch 1 computes the coordination numbers cn per atom on device.
  - The host applies the static edge->atom join (gathers cn[j] into a per-edge
    stream slot) and launch 2 computes weights, the C6 bilinear term and the
    damped energy per edge, segment-reduces per atom, dots with the i-atom
    weights and reduces to one scalar per core.  The host sums the 8 partial
    scalars (the "all-reduce").
  - All static per-edge element data (rcov/r4r2/ref_cn rows and the 5x5
    ref_c6 block, bf16) is host-gathered into the edge streams; all floating
    point math happens on device.
"""

import sys

sys.path.insert(0, "/opt/trn_rl_repo")

from contextlib import ExitStack

import ml_dtypes
import numpy as np

import concourse.bacc as bacc
import concourse.bass as bass
import concourse.mybir as mybir
import concourse.tile as tile
from concourse.bass_utils import run_bass_kernel_spmd

F32 = mybir.dt.float32
BF16 = mybir.dt.bfloat16
AF = mybir.ActivationFunctionType
ALU = mybir.AluOpType
AX = mybir.AxisListType

BOHR = 0.5291772105638411
HA = 27.211386024367243
S6, S8, A1, A2 = 1.0, 0.7875, 0.4289, 4.4407
KCN = 16.0
WF = 4.0
EPS32 = float(np.finfo(np.float32).eps)

NCORES = 8
P = 128
# degree buckets (pad each atom's edge run up to the next bucket length)
LS = [8, 16, 24, 32, 40, 48, 64, 96, 128, 192, 256, 384]
MAXCOLS = 576  # max slot columns per partition per piece

SLOT1 = 4  # launch-1 stream f32 lanes: dx dy dz rcov_j
SLOT2 = 10  # launch-2 stream bf16 lanes: dx dy dz r4r2_j ref_j[5] pad

_cache = {}
REPEAT = 1
TRACE = False
LAST_HW_NS = None
LAST_R1 = None
LAST_R2 = None


def _build_geometry(counts, atom_ranges):
    """Shared (all-core) piece geometry from per-core degree histograms."""
    ncore = len(atom_ranges)
    # per-core atoms per bucket
    percore = []
    for a0, a1 in atom_ranges:
        degs = counts[a0:a1]
        li = np.searchsorted(LS, degs, side="left")
        assert li.max() < len(LS), f"degree {degs.max()} exceeds bucket table"
        percore.append(np.bincount(li, minlength=len(LS)))
    nmax = np.stack(percore).max(axis=0)  # atoms per bucket, unified
    # pad atom count per bucket to a multiple of P
    nmax = ((nmax + P - 1) // P) * P

    pieces = []  # (L, n_p, scol_off, acol_off)
    group_info = []  # per bucket: (L, n_atoms, scol_off, acol_off)
    scol = 0
    acol = 0
    for bi, L in enumerate(LS):
        n = int(nmax[bi])
        if n == 0:
            group_info.append((L, 0, scol, acol))
            continue
        n_cols = n // P
        group_info.append((L, n, scol, acol))
        npp = max(1, MAXCOLS // L)
        c = 0
        while c < n_cols:
            take = min(npp, n_cols - c)
            pieces.append((L, take, scol + c * L, acol + c))
            c += take
        scol += n_cols * L
        acol += n_cols
    return pieces, group_info, scol, acol


def _prep(dr_vec, ref_cn_table, ref_c6_table, r4r2_table, rcov_table, numbers, idx):
    N = numbers.shape[0]
    E = idx.shape[1]
    i = idx[0].astype(np.int64)
    j = idx[1].astype(np.int64)

    counts = np.bincount(i, minlength=N)
    ccum = np.concatenate([[0], np.cumsum(counts)])
    # atom-aligned shard boundaries, balanced by edge count
    targets = [E * k // NCORES for k in range(1, NCORES)]
    cuts = [0] + [int(np.searchsorted(ccum, t)) for t in targets] + [N]
    atom_ranges = [(cuts[k], cuts[k + 1]) for k in range(NCORES)]

    pieces, groups, COLS, ACOLS = _build_geometry(counts, atom_ranges)

    order = np.argsort(i, kind="stable")
    i_s = i[order]
    pos = np.arange(E, dtype=np.int64) - ccum[i_s]  # rank of edge within its atom run

    # static per-edge element data (host gathers of input tables, no arithmetic)
    Zi = numbers[i].astype(np.int64)
    Zj = numbers[j].astype(np.int64)
    rcov_a = rcov_table[numbers]
    r4r2_a = r4r2_table[numbers]

    bf = ref_c6_table[Zj, Zi].reshape(E, 25).astype(ml_dtypes.bfloat16)

    cores = []
    for k, (a0, a1) in enumerate(atom_ranges):
        nloc = a1 - a0
        degs = counts[a0:a1]
        li = np.searchsorted(LS, degs, side="left")
        # per-atom placement: within its bucket group, atoms sorted by id
        part = np.empty(nloc, np.int64)
        acol_of = np.empty(nloc, np.int64)
        scolb = np.empty(nloc, np.int64)
        agrid = np.full((P, ACOLS), -1, np.int64)
        for bi, (L, n, scol0, acol0) in enumerate(groups):
            sel = np.nonzero(li == bi)[0]  # local atom indices, ascending
            if len(sel) == 0:
                continue
            t = np.arange(len(sel))
            c = t // P
            p = t % P
            part[sel] = p
            acol_of[sel] = acol0 + c
            scolb[sel] = scol0 + c * L
            agrid[p, acol0 + c] = sel + a0

        e0, e1 = ccum[a0], ccum[a1]
        eo = order[e0:e1]  # global edge ids of this core, i-sorted
        il = i_s[e0:e1] - a0  # local i atom
        pp = part[il]
        cc = scolb[il] + pos[e0:e1]

        s1 = np.zeros((P, COLS, SLOT1), np.float32)
        s1[pp, cc, 0] = dr_vec[eo, 0]
        s1[pp, cc, 1] = dr_vec[eo, 1]
        s1[pp, cc, 2] = dr_vec[eo, 2]
        s1[pp, cc, 3] = rcov_a[j[eo]]

        s2 = np.zeros((P, COLS, SLOT2), ml_dtypes.bfloat16)
        s2[pp, cc, 0] = dr_vec[eo, 0]
        s2[pp, cc, 1] = dr_vec[eo, 1]
        s2[pp, cc, 2] = dr_vec[eo, 2]
        s2[pp, cc, 3] = r4r2_a[j[eo]]
        s2[pp, cc, 4:9] = ref_cn_table[Zj[eo]]
        scn = np.zeros((P, COLS), np.float32)
        s2b = np.zeros((P, 25, COLS), ml_dtypes.bfloat16)
        s2b[pp[:, None], np.arange(25)[None, :], cc[:, None]] = bf[eo]

        at1 = np.zeros((P, ACOLS), np.float32)
        at2 = np.zeros((P, ACOLS, 8), np.float32)
        am = agrid >= 0
        at1[am] = rcov_a[agrid[am]]
        at2[am, 0] = r4r2_a[agrid[am]]
        at2[am, 1:6] = ref_cn_table[numbers[agrid[am]]]

        cores.append(
            dict(s1=s1, s2=s2, s2b=s2b, scn=scn, at1=at1, at2=at2, agrid=agrid,
                 pp=pp, cc=cc, jglob=j[eo])
        )

    return dict(
        pieces=pieces, COLS=COLS, ACOLS=ACOLS, cores=cores, N=N, E=E,
    )


def _new_nc():
    return bacc.Bacc("TRN2", target_bir_lowering=False, debug=False, num_devices=NCORES)


def _build_l1(pieces, COLS, ACOLS):
    nc = _new_nc()
    s1 = nc.declare_dram_parameter("s1", [P, COLS * SLOT1], F32, isOutput=False)
    at1 = nc.declare_dram_parameter("at1", [P, ACOLS], F32, isOutput=False)
    cno = nc.declare_dram_parameter("cn", [P, ACOLS], F32, isOutput=True)

    with ExitStack() as ctx:
        tc = ctx.enter_context(tile.TileContext(nc))
        persist = ctx.enter_context(tc.tile_pool(name="persist", bufs=1))
        spool = ctx.enter_context(tc.tile_pool(name="stream", bufs=2))
        wpool = ctx.enter_context(tc.tile_pool(name="work", bufs=2))

        cn_t = persist.tile([P, ACOLS], F32)
        at_t = persist.tile([P, ACOLS], F32)
        nc.sync.dma_start(at_t[:], at1[:])
        b_tiny = persist.tile([P, 1], F32)
        nc.vector.memset(b_tiny[:], 1e-30)
        b_negk = persist.tile([P, 1], F32)
        nc.vector.memset(b_negk[:], -KCN)

        for _rep in range(REPEAT):
          for (L, n_p, scol, acol) in pieces:
            W = n_p * L
            st = spool.tile([P, W * SLOT1], F32, tag="st")
            nc.sync.dma_start(st[:], s1[:, scol * SLOT1:(scol + W) * SLOT1])
            v = st[:].rearrange("p (a l f) -> p a l f", a=n_p, l=L, f=SLOT1)
            dx, dy, dz, rcj = (v[:, :, :, q] for q in range(4))

            s = wpool.tile([P, n_p, L], F32, tag="s")
            t = wpool.tile([P, n_p, L], F32, tag="t")
            nc.vector.tensor_tensor(s[:], dx, dx, ALU.mult)
            nc.vector.tensor_tensor(t[:], dy, dy, ALU.mult)
            nc.vector.tensor_tensor(s[:], s[:], t[:], ALU.add)
            nc.vector.tensor_tensor(t[:], dz, dz, ALU.mult)
            nc.vector.tensor_tensor(s[:], s[:], t[:], ALU.add)
            dr = wpool.tile([P, n_p, L], F32, tag="dr")
            # dr = sqrt(|d|^2/BOHR^2 + tiny); tiny keeps pad slots finite
            nc.scalar.activation(dr[:], s[:], AF.Sqrt, scale=1.0 / BOHR**2, bias=b_tiny[:])
            rdr = wpool.tile([P, n_p, L], F32, tag="rdr")
            nc.vector.reciprocal(rdr[:], dr[:])
            rc = wpool.tile([P, n_p, L], F32, tag="rc")
            rci = at_t[:, acol:acol + n_p].unsqueeze(-1).to_broadcast([P, n_p, L])
            nc.vector.tensor_tensor(rc[:], rcj, rci, ALU.add)
            targ = wpool.tile([P, n_p, L], F32, tag="targ")
            nc.vector.tensor_tensor(targ[:], rc[:], rdr[:], ALU.mult)
            cnt = wpool.tile([P, n_p, L], F32, tag="cnt")
            nc.scalar.activation(cnt[:], targ[:], AF.Sigmoid, scale=KCN, bias=b_negk[:])
            mcn = wpool.tile([P, n_p, L], F32, tag="mcn")
            # mcn = (dx2sum > 0) * count   (pad slots have |d|^2 == 0)
            nc.vector.scalar_tensor_tensor(mcn[:], s[:], 0.0, cnt[:], ALU.is_gt, ALU.mult)
            nc.vector.tensor_reduce(cn_t[:, acol:acol + n_p], mcn[:], AX.X, ALU.add)

        nc.sync.dma_start(cno[:], cn_t[:])
    nc.compile()
    return nc


def _build_l2(pieces, COLS, ACOLS):
    import os
    _skip = set(os.environ.get("L2SKIP", "").split(","))
    nc = _new_nc()
    s2 = nc.declare_dram_parameter("s2", [P, COLS * SLOT2], BF16, isOutput=False)
    scn = nc.declare_dram_parameter("scn", [P, COLS], F32, isOutput=False)
    s2b = nc.declare_dram_parameter("s2b", [P, 25 * COLS], BF16, isOutput=False)
    at2 = nc.declare_dram_parameter("at2", [P, ACOLS * 8], F32, isOutput=False)
    cni = nc.declare_dram_parameter("cn", [P, ACOLS], F32, isOutput=False)
    eto = nc.declare_dram_parameter("etot", [1, 1], F32, isOutput=True)
    s2bv = s2b[:].rearrange("p (m c) -> p m c", m=25)

    with ExitStack() as ctx:
        tc = ctx.enter_context(tile.TileContext(nc))
        persist = ctx.enter_context(tc.tile_pool(name="persist", bufs=1))
        spool = ctx.enter_context(tc.tile_pool(name="stream", bufs=2))
        wpool = ctx.enter_context(tc.tile_pool(name="work", bufs=2))
        w5pool = ctx.enter_context(tc.tile_pool(name="work5", bufs=2))
        bpool = ctx.enter_context(tc.tile_pool(name="workb", bufs=2))
        ppool = ctx.enter_context(tc.tile_pool(name="psum", bufs=1, space="PSUM"))

        att = persist.tile([P, ACOLS, 8], F32)
        nc.sync.dma_start(att[:], at2[:])
        cnt_ = persist.tile([P, ACOLS], F32)
        nc.sync.dma_start(cnt_[:], cni[:])
        b_a2 = persist.tile([P, 1], F32)
        nc.vector.memset(b_a2[:], A2)

        # ---- per-atom weights, plane-major: w5p[P, 5, ACOLS]
        w5p = persist.tile([P, 5, ACOLS], F32)
        attv = att[:].rearrange("p a f -> p f a")  # [P, 8, ACOLS] strided view
        nc.vector.tensor_tensor(
            w5p[:], attv[:, 1:6, :], cnt_[:].unsqueeze(1).to_broadcast([P, 5, ACOLS]),
            ALU.subtract,
        )
        sq = persist.tile([P, 5, ACOLS], F32)
        nc.scalar.activation(sq[:], w5p[:], AF.Square)
        nc.scalar.activation(w5p[:], sq[:], AF.Exp, scale=-WF)
        wsum = persist.tile([P, ACOLS], F32)
        nc.vector.tensor_tensor(wsum[:], w5p[:, 0, :], w5p[:, 1, :], ALU.add)
        nc.vector.tensor_tensor(wsum[:], wsum[:], w5p[:, 2, :], ALU.add)
        nc.vector.tensor_tensor(wsum[:], wsum[:], w5p[:, 3, :], ALU.add)
        nc.vector.tensor_tensor(wsum[:], wsum[:], w5p[:, 4, :], ALU.add)
        nc.vector.tensor_scalar_add(wsum[:], wsum[:], EPS32)
        winv = persist.tile([P, ACOLS], F32)
        nc.vector.reciprocal(winv[:], wsum[:])
        nc.vector.tensor_scalar_mul(winv[:], winv[:], -HA / 2.0)
        nc.vector.tensor_tensor(
            w5p[:], w5p[:], winv[:].unsqueeze(1).to_broadcast([P, 5, ACOLS]), ALU.mult
        )
        r43 = persist.tile([P, ACOLS], F32)
        nc.vector.tensor_scalar_mul(r43[:], att[:, :, 0], 3.0)

        # ---- per-edge pieces
        ecols = []
        for _rep in range(REPEAT):
          for pi, (L, n_p, scol, acol) in enumerate(pieces):
            W = n_p * L
            st = spool.tile([P, W * SLOT2], BF16, tag="st")
            sc = spool.tile([P, W], F32, tag="sc")
            sb = spool.tile([P, 25 * W], BF16, tag="sb")
            if "dma" not in _skip:
                nc.sync.dma_start(st[:], s2[:, scol * SLOT2:(scol + W) * SLOT2])
                nc.sync.dma_start(sc[:], scn[:, scol:scol + W])
                nc.sync.dma_start(
                    sb[:].rearrange("p (m w) -> p m w", m=25), s2bv[:, :, scol:scol + W]
                )
            else:
                nc.gpsimd.memset(st[:], 0.0)
                nc.gpsimd.memset(sc[:], 0.0)
                nc.gpsimd.memset(sb[:], 0.0)
            vp = st[:].rearrange("p (w f) -> p f w", f=SLOT2)  # [P,10,W] bf16 strided
            v3 = st[:].rearrange("p (a l f) -> p a l f", a=n_p, l=L, f=SLOT2)
            mb = sb[:].rearrange("p (m w) -> p m w", m=25)  # [P,25,W] bf16

            def wt(tag):
                return wpool.tile([P, W], F32, tag=tag, name=tag)

            # |d|^2 via ACT squares + DVE adds (tags t0..t5 manually recycled)
            if "dr" in _skip:
                D = wt("t0")
                nc.gpsimd.memset(D[:], 0.0)
            tx = wt("t0")
            ty = wt("t1")
            tz = wt("t2")
            if "dr" not in _skip:
              def bt(tag):
                  return bpool.tile([P, W], BF16, tag=tag, name=tag)

              tx = bt("b0")
              ty = bt("b1")
              tz = bt("b2")
              nc.scalar.activation(tx[:], vp[:, 0, :], AF.Square)
              nc.scalar.activation(ty[:], vp[:, 1, :], AF.Square)
              nc.scalar.activation(tz[:], vp[:, 2, :], AF.Square)
              s_ = bt("b3")
              nc.vector.tensor_tensor(s_[:], tx[:], ty[:], ALU.add)
              nc.vector.tensor_tensor(s_[:], s_[:], tz[:], ALU.add)
              t_ = bt("b0")
              nc.scalar.activation(t_[:], s_[:], AF.Square, scale=1.0 / BOHR**2)
              dr6 = bt("b1")
              nc.vector.scalar_tensor_tensor(
                  dr6[:], t_[:], 1.0 / BOHR**2, s_[:], ALU.mult, ALU.mult
              )
              dr8 = bt("b2")
              nc.vector.scalar_tensor_tensor(
                  dr8[:], dr6[:], 1.0 / BOHR**2, s_[:], ALU.mult, ALU.mult
              )
              qq = wt("t0")
              r4ib = r43[:, acol:acol + n_p].unsqueeze(-1).to_broadcast([P, n_p, L])
              nc.vector.tensor_tensor(
                  qq[:].rearrange("p (a l) -> p a l", a=n_p), v3[:, :, :, 3], r4ib,
                  ALU.mult,
              )
              rrs = bt("b3")
              nc.scalar.activation(rrs[:], qq[:], AF.Sqrt, scale=A1 * A1)
              rr2 = bt("b4")
              nc.scalar.activation(rr2[:], rrs[:], AF.Square, bias=b_a2[:])
              t2_ = bt("b3")
              nc.scalar.activation(t2_[:], rr2[:], AF.Square)
              rr6 = bt("b5")
              nc.vector.tensor_tensor(rr6[:], t2_[:], rr2[:], ALU.mult)
              nc.vector.tensor_tensor(dr6[:], dr6[:], rr6[:], ALU.add)  # den6
              i6 = wt("t1")
              nc.vector.reciprocal(i6[:], dr6[:])
              nc.vector.tensor_tensor(rr6[:], rr6[:], rr2[:], ALU.mult)  # rr8
              nc.vector.tensor_tensor(dr8[:], dr8[:], rr6[:], ALU.add)  # den8
              i8 = wt("t2")
              nc.vector.reciprocal(i8[:], dr8[:])
              t8 = wt("t3")
              nc.vector.tensor_tensor(t8[:], qq[:], i8[:], ALU.mult)
              D = wt("t0")
              nc.vector.scalar_tensor_tensor(D[:], i6[:], S6 / S8, t8[:], ALU.mult, ALU.add)

            # vj planes: f32 sub -> ACT Square (in place) -> ACT Exp -> bf16
            vjf = w5pool.tile([P, 5, W], F32, tag="vjf")
            vj = bpool.tile([P, 5, W], BF16, tag="vj")
            wjs = bpool.tile([P, W], BF16, tag="wjs")
            wji = wt("t2")
            Dw = wt("t3")
            if "vj" in _skip:
                nc.gpsimd.memset(vj[:], 0.0)
                nc.gpsimd.memset(Dw[:], 0.0)
            if "vj" not in _skip:
              nc.vector.tensor_tensor(
                vjf[:], vp[:, 4:9, :],
                sc[:].unsqueeze(1).to_broadcast([P, 5, W]), ALU.subtract,
              )
              nc.scalar.activation(vjf[:], vjf[:], AF.Square)
              vj2 = None
              nc.scalar.activation(vj[:], vjf[:], AF.Exp, scale=-WF)
              nc.vector.tensor_tensor(wjs[:], vj[:, 0, :], vj[:, 1, :], ALU.add)
              nc.vector.tensor_tensor(wjs[:], wjs[:], vj[:, 2, :], ALU.add)
              nc.vector.tensor_tensor(wjs[:], wjs[:], vj[:, 3, :], ALU.add)
              nc.vector.tensor_tensor(wjs[:], wjs[:], vj[:, 4, :], ALU.add)
              nc.vector.tensor_scalar_add(wjs[:], wjs[:], EPS32)
              nc.vector.reciprocal(wji[:], wjs[:])
              nc.vector.scalar_tensor_tensor(Dw[:], D[:], S8, wji[:], ALU.mult, ALU.mult)

            # z[s] = sum_r M[5r+s] * vj[r]  (bf16 2x), then zD = z * Dw
            z = bpool.tile([P, 5, W], BF16, tag="z")
            _doc6 = "c6" not in _skip
            if not _doc6:
                nc.gpsimd.memset(z[:], 0.0)
            tmpb = bpool.tile([P, W], BF16, tag="tmpb")
            import os
            ngp = int(os.environ.get("GPOFF", "0"))
            tmpg = bpool.tile([P, W], BF16, tag="tmpg") if ngp else None
            for si in range(5) if _doc6 else []:
                eng = nc.gpsimd if si >= 5 - ngp else nc.vector
                tb = tmpg if si >= 5 - ngp else tmpb
                zs = z[:, si, :]
                eng.tensor_tensor(zs, mb[:, si, :], vj[:, 0, :], ALU.mult)
                for r in range(1, 5):
                    eng.tensor_tensor(
                        tb[:], mb[:, 5 * r + si, :], vj[:, r, :], ALU.mult
                    )
                    eng.tensor_tensor(zs, zs, tb[:], ALU.add)
            Dwb = bpool.tile([P, W], BF16, tag="Dwb")
            if _doc6:
                nc.vector.tensor_copy(Dwb[:], Dw[:])
                nc.vector.tensor_tensor(
                    z[:], z[:], Dwb[:].unsqueeze(1).to_broadcast([P, 5, W]), ALU.mult
                )
            Sp = w5pool.tile([P, 5, n_p], F32, tag="Sp")
            nc.vector.tensor_reduce(
                Sp[:], z[:].rearrange("p s (a l) -> p s a l", a=n_p), AX.X, ALU.add
            )

            junk = w5pool.tile([P, 5, n_p], F32, tag="junk")
            nc.vector.tensor_tensor(
                junk[:], Sp[:], w5p[:, :, acol:acol + n_p], ALU.mult
            )
            ep = persist.tile([P, 1], F32, tag="ep", name="ep")
            nc.vector.tensor_reduce(ep[:], junk[:], AX.XY, ALU.add)
            if pi == 0:
                eacc = persist.tile([P, 1], F32, name="eacc", tag="eacc")
                ecols = [eacc]
                nc.vector.tensor_copy(eacc[:], ep[:])
            else:
                nc.vector.tensor_tensor(ecols[0][:], ecols[0][:], ep[:], ALU.add)

        ones = persist.tile([P, 1], F32)
        nc.vector.memset(ones[:], 1.0)
        ps = ppool.tile([1, 1], F32)
        nc.tensor.matmul(ps[:], ones[:], ecols[-1][:], start=True, stop=True)
        esb = persist.tile([1, 1], F32)
        nc.scalar.copy(esb[:], ps[:])
        nc.sync.dma_start(eto[:], esb[:])
    nc.compile()
    return nc


def kernel(dr_vec, ref_cn_table, ref_c6_table, r4r2_table, rcov_table, numbers, idx):
    # smooth_cutoff(dr, 20, 25) and (55, 60) are identically 1 for this data
    assert np.sqrt((dr_vec.astype(np.float64) ** 2).sum(-1)).max() / BOHR < 19.0
    prep = _prep(dr_vec, ref_cn_table, ref_c6_table, r4r2_table, rcov_table, numbers, idx)
    pieces, COLS, ACOLS = prep["pieces"], prep["COLS"], prep["ACOLS"]

    key = (tuple(pieces), COLS, ACOLS)
    if key not in _cache:
        _cache[key] = (_build_l1(pieces, COLS, ACOLS), _build_l2(pieces, COLS, ACOLS))
    nc1, nc2 = _cache[key]

    in1 = [
        {"s1": c["s1"].reshape(P, -1), "at1": c["at1"]} for c in prep["cores"]
    ]
    global LAST_HW_NS, LAST_R1, LAST_R2
    r1 = run_bass_kernel_spmd(nc1, in1, list(range(NCORES)), trace=TRACE)

    N = prep["N"]
    cn_full = np.zeros(N, np.float32)
    for k, c in enumerate(prep["cores"]):
        cn_k = r1.results[k]["cn"]
        m = c["agrid"] >= 0
        cn_full[c["agrid"][m]] = cn_k[m]

    in2 = []
    for k, c in enumerate(prep["cores"]):
        c["scn"][c["pp"], c["cc"]] = cn_full[c["jglob"]]
        in2.append(
            {
                "s2": c["s2"].reshape(P, -1),
                "s2b": c["s2b"].reshape(P, -1),
                "scn": c["scn"],
                "at2": c["at2"].reshape(P, -1),
                "cn": r1.results[k]["cn"],
            }
        )
    r2 = run_bass_kernel_spmd(nc2, in2, list(range(NCORES)), trace=TRACE)
    LAST_R1, LAST_R2 = r1, r2
    if TRACE and r1.exec_time_ns and r2.exec_time_ns:
        LAST_HW_NS = r1.exec_time_ns + r2.exec_time_ns

    parts = [r2.results[k]["etot"].reshape(()) for k in range(NCORES)]
    return np.float32(np.sum(np.stack(parts)))



# revision 8
# speedup vs baseline: 2.9493x; 2.9493x over previous
"""Trainium2 Bass kernel for the DispaxD3 two-body dispersion energy.

Two SPMD launches over 8 cores:

L1 (edges sharded by i-atom, degree-bucketed layout [128, n_p, L]):
  per edge: dr from dx,dy,dz; coordination-number count (segment-summed
  per atom); BJ damping factor D = S6/S8/(dr6+rr6) + qq/(dr8+rr8) (bf16 out).
  per atom: gaussian reference weights w[5] from cn (bf16 out).

Host join (gathers/permutations only, no arithmetic): assemble per-atom
w, permute per-edge D into the pair-sorted layout, gather w[i]/w[j] per
edge.

L2 (edges sharded + sorted by element pair (Zj,Zi), chunks of 128 edges
on partitions):
  DVE: a_r = D*wj_r (5 bf16 TTs).
  PE : per 8-chunk group, matmul lhsT=a-block [128,(5r,8c)] x
       rhs=wi-block [128,(5s,8c)] -> psum [40,40]; diagonal c==c' blocks
       hold G[chunk,r,s] = sum_k D*wj_r*wi_s.  The 5x5 C6 block of each
       PAIR is applied via a host-built mask table (C6 at diagonal
       positions, zero elsewhere), so ref_c6 is loaded once per pair,
       not once per edge.
  ACT: drains psum -> bf16; DVE: prod = G*mask; PE: ones-colsum
       accumulates everything into one [1,480] psum; final scale by
       -S8*HA/2 on device.
"""

import sys

sys.path.insert(0, "/opt/trn_rl_repo")

from contextlib import ExitStack

import ml_dtypes
import numpy as np

import concourse.bacc as bacc
import concourse.bass as bass
import concourse.mybir as mybir
import concourse.tile as tile
from concourse.bass_utils import run_bass_kernel_spmd

F32 = mybir.dt.float32
BF16 = mybir.dt.bfloat16
AF = mybir.ActivationFunctionType
ALU = mybir.AluOpType
AX = mybir.AxisListType

BOHR = 0.5291772105638411
HA = 27.211386024367243
S6, S8, A1, A2 = 1.0, 0.7875, 0.4289, 4.4407
KCN = 16.0
WF = 4.0
EPS32 = float(np.finfo(np.float32).eps)
NELEM = 95

NCORES = 8
P = 128
LS = [8, 16, 24, 32, 40, 48, 64, 96, 128, 192, 256, 384]
MAXCOLS = 1152  # L1 piece width budget (slot columns per partition)

SLOT1 = 4  # L1 f32 lanes: dx dy dz rcov_j
# L2 grouping: 8 chunks/group, 12 groups/round (psum bank), 6 rounds/tile
GCH = 8
GPR = 12
RPT = 6
TCH = GCH * GPR * RPT  # 576 chunks per tile

_cache = {}
REPEAT = 1
TRACE = False


# ---------------------------------------------------------------- geometry
def _build_geometry(counts, atom_ranges):
    percore = []
    for a0, a1 in atom_ranges:
        degs = counts[a0:a1]
        li = np.searchsorted(LS, degs, side="left")
        assert li.max() < len(LS), f"degree {degs.max()} exceeds bucket table"
        percore.append(np.bincount(li, minlength=len(LS)))
    nmax = np.stack(percore).max(axis=0)
    nmax = ((nmax + P - 1) // P) * P

    pieces = []
    group_info = []
    scol = 0
    acol = 0
    for bi, L in enumerate(LS):
        n = int(nmax[bi])
        if n == 0:
            group_info.append((L, 0, scol, acol))
            continue
        n_cols = n // P
        group_info.append((L, n, scol, acol))
        npp = max(1, MAXCOLS // L)
        c = 0
        while c < n_cols:
            take = min(npp, n_cols - c)
            pieces.append((L, take, scol + c * L, acol + c))
            c += take
        scol += n_cols * L
        acol += n_cols
    return pieces, group_info, scol, acol


def _prep(dr_vec, ref_cn_table, ref_c6_table, r4r2_table, rcov_table, numbers, idx):
    N = numbers.shape[0]
    E = idx.shape[1]
    i = idx[0].astype(np.int64)
    j = idx[1].astype(np.int64)

    # ---------------- L1: shard edges by i-atom at atom boundaries
    counts = np.bincount(i, minlength=N)
    ccum = np.concatenate([[0], np.cumsum(counts)])
    targets = [E * k // NCORES for k in range(1, NCORES)]
    cuts = [0] + [int(np.searchsorted(ccum, t)) for t in targets] + [N]
    atom_ranges = [(cuts[k], cuts[k + 1]) for k in range(NCORES)]

    pieces, groups, COLS, ACOLS = _build_geometry(counts, atom_ranges)

    order = np.argsort(i, kind="stable")
    i_s = i[order]
    pos = np.arange(E, dtype=np.int64) - ccum[i_s]

    rcov_a = rcov_table[numbers]
    r4r2_a = r4r2_table[numbers]

    # per-edge L1 slot (core, partition, col), indexed by global edge id
    e_core = np.empty(E, np.int64)
    e_pp1 = np.empty(E, np.int64)
    e_cc1 = np.empty(E, np.int64)

    cores = []
    for k, (a0, a1) in enumerate(atom_ranges):
        nloc = a1 - a0
        degs = counts[a0:a1]
        li = np.searchsorted(LS, degs, side="left")
        part = np.empty(nloc, np.int64)
        acol_of = np.empty(nloc, np.int64)
        scolb = np.empty(nloc, np.int64)
        agrid = np.full((P, ACOLS), -1, np.int64)
        for bi, (L, n, scol0, acol0) in enumerate(groups):
            sel = np.nonzero(li == bi)[0]
            if len(sel) == 0:
                continue
            t = np.arange(len(sel))
            c = t // P
            p = t % P
            part[sel] = p
            acol_of[sel] = acol0 + c
            scolb[sel] = scol0 + c * L
            agrid[p, acol0 + c] = sel + a0

        e0, e1 = ccum[a0], ccum[a1]
        eo = order[e0:e1]
        il = i_s[e0:e1] - a0
        pp = part[il]
        cc = scolb[il] + pos[e0:e1]
        e_core[eo] = k
        e_pp1[eo] = pp
        e_cc1[eo] = cc

        s1 = np.zeros((P, COLS, SLOT1), np.float32)
        s1[:, :, 3] = -1e4  # pad slots: rc<0 kills the sigmoid count
        s1[pp, cc, 0] = dr_vec[eo, 0]
        s1[pp, cc, 1] = dr_vec[eo, 1]
        s1[pp, cc, 2] = dr_vec[eo, 2]
        s1[pp, cc, 3] = rcov_a[j[eo]]
        s1h = np.zeros((P, COLS), ml_dtypes.bfloat16)
        s1h[pp, cc] = r4r2_a[j[eo]].astype(ml_dtypes.bfloat16)

        at2 = np.zeros((P, ACOLS, 8), np.float32)
        am = agrid >= 0
        at2[am, 0] = r4r2_a[agrid[am]]
        at2[am, 1:6] = ref_cn_table[numbers[agrid[am]]]
        at2[am, 6] = rcov_a[agrid[am]]

        cores.append(dict(s1=s1, s1h=s1h, at2=at2, agrid=agrid))

    # ---------------- L2: shard + sort edges by element pair (Zj, Zi)
    Zi = numbers[i].astype(np.int64)
    Zj = numbers[j].astype(np.int64)
    pid = Zj * NELEM + Zi
    order2 = np.argsort(pid, kind="stable")
    pid_s = pid[order2]
    pcnt = np.bincount(pid, minlength=NELEM * NELEM)
    pcum = np.concatenate([[0], np.cumsum(pcnt)])
    # shard contiguous pair ranges balanced by edge count
    t2 = [E * k // NCORES for k in range(1, NCORES)]
    pc = [0] + [int(np.searchsorted(pcum, t)) for t in t2] + [NELEM * NELEM]
    pair_ranges = [(pc[k], pc[k + 1]) for k in range(NCORES)]

    rank = np.arange(E, dtype=np.int64) - pcum[pid_s]  # rank of edge in its pair

    core2 = []
    nch_list = []
    for k, (p0, p1) in enumerate(pair_ranges):
        cnts = pcnt[p0:p1]
        nchunks = (cnts + P - 1) // P
        cbase = np.concatenate([[0], np.cumsum(nchunks)])
        nch = int(cbase[-1])
        nch_list.append(nch)
        e0, e1 = pcum[p0], pcum[p1]
        eo2 = order2[e0:e1]
        rk = rank[e0:e1]
        pl = pid_s[e0:e1] - p0  # local pair index
        pp2 = rk % P
        cc2 = cbase[pl] + rk // P
        # chunk -> pair map
        chpair = np.repeat(np.arange(p1 - p0), nchunks) + p0
        core2.append(dict(eo2=eo2, pp2=pp2, cc2=cc2, chpair=chpair, nch=nch))

    NCHP = ((max(nch_list) + TCH - 1) // TCH) * TCH
    RND = NCHP // (GCH * GPR)

    # mask tables: C6 at diagonal (c==c') block positions
    for k in range(NCORES):
        c2 = core2[k]
        nch = c2["nch"]
        chp = c2["chpair"]
        zj = chp // NELEM
        zi = chp % NELEM
        cvals = ref_c6_table[zj, zi].astype(np.float32)  # [nch, 5, 5]
        ch = np.arange(nch)
        rd = ch // (GCH * GPR)
        gg = (ch % (GCH * GPR)) // GCH
        cc = ch % GCH
        mask = np.zeros((5 * GCH, RND * GPR * 5 * GCH), ml_dtypes.bfloat16)
        r_ = np.arange(5)
        s_ = np.arange(5)
        rows = (r_[None, :, None] * GCH + cc[:, None, None]) * np.ones(
            (1, 1, 5), np.int64
        )
        colbase = rd * (GPR * 5 * GCH) + gg * (5 * GCH)
        cols = (
            colbase[:, None, None] + s_[None, None, :] * GCH + cc[:, None, None]
        ) * np.ones((1, 5, 1), np.int64)
        mask[rows.reshape(-1), cols.reshape(-1)] = cvals.reshape(-1).astype(
            ml_dtypes.bfloat16
        )
        c2["mask"] = mask

    return dict(
        pieces=pieces, COLS=COLS, ACOLS=ACOLS, cores=cores, core2=core2,
        N=N, E=E, NCHP=NCHP, RND=RND,
        e_core=e_core, e_pp1=e_pp1, e_cc1=e_cc1, i=i, j=j,
    )


def _new_nc():
    return bacc.Bacc("TRN2", target_bir_lowering=False, debug=False, num_devices=NCORES)


# ---------------------------------------------------------------- launch 1
def _build_l1(pieces, COLS, ACOLS):
    nc = _new_nc()
    s1 = nc.declare_dram_parameter("s1", [P, COLS * SLOT1], F32, isOutput=False)
    s1h = nc.declare_dram_parameter("s1h", [P, COLS], BF16, isOutput=False)
    at2 = nc.declare_dram_parameter("at2", [P, ACOLS * 8], F32, isOutput=False)
    dout = nc.declare_dram_parameter("D", [P, COLS], BF16, isOutput=True)
    wout = nc.declare_dram_parameter("w", [P, 5 * ACOLS], BF16, isOutput=True)

    RB = 1.0 / BOHR / BOHR

    with ExitStack() as ctx:
        tc = ctx.enter_context(tile.TileContext(nc))
        persist = ctx.enter_context(tc.tile_pool(name="persist", bufs=1))
        spool = ctx.enter_context(tc.tile_pool(name="stream", bufs=2))
        wpool = ctx.enter_context(tc.tile_pool(name="work", bufs=2))
        bpool = ctx.enter_context(tc.tile_pool(name="workb", bufs=2))

        att = persist.tile([P, ACOLS, 8], F32)
        nc.sync.dma_start(att[:], at2[:])
        cn_t = persist.tile([P, ACOLS], F32)
        D_t = persist.tile([P, COLS], BF16)
        b_tiny = persist.tile([P, 1], F32)
        nc.vector.memset(b_tiny[:], 1e-30)
        b_negk = persist.tile([P, 1], F32)
        nc.vector.memset(b_negk[:], -KCN)
        b_a2 = persist.tile([P, 1], F32)
        nc.vector.memset(b_a2[:], A2)
        r43 = persist.tile([P, ACOLS], F32)
        nc.vector.tensor_scalar_mul(r43[:], att[:, :, 0], 3.0)

        for _rep in range(REPEAT):
          for (L, n_p, scol, acol) in pieces:
            W = n_p * L
            st = spool.tile([P, W * SLOT1], F32, tag="st")
            nc.sync.dma_start(st[:], s1[:, scol * SLOT1:(scol + W) * SLOT1])
            sh = spool.tile([P, W], BF16, tag="sh")
            nc.sync.dma_start(sh[:], s1h[:, scol:scol + W])
            v = st[:].rearrange("p (a l f) -> p a l f", a=n_p, l=L, f=SLOT1)
            dx, dy, dz, rcj = (v[:, :, :, q] for q in range(4))

            def wt(tag):
                return wpool.tile([P, W], F32, tag=tag, name=tag)

            def bt(tag):
                return bpool.tile([P, W], BF16, tag=tag, name=tag)

            tx = wt("t0")
            ty = wt("t1")
            nc.scalar.activation(tx[:], dx, AF.Square)
            nc.scalar.activation(ty[:], dy, AF.Square)
            s2 = wt("t2")
            nc.vector.tensor_tensor(s2[:], tx[:], ty[:], ALU.add)
            nc.scalar.activation(tx[:], dz, AF.Square)
            nc.vector.tensor_tensor(s2[:], s2[:], tx[:], ALU.add)
            # ---- coordination-number count
            dr = wt("t0")
            nc.scalar.activation(dr[:], s2[:], AF.Sqrt, scale=RB, bias=b_tiny[:])
            rdr = wt("t1")
            nc.vector.reciprocal(rdr[:], dr[:])
            rc = wt("t3")
            rci = att[:, acol:acol + n_p, 6].unsqueeze(-1).to_broadcast([P, n_p, L])
            nc.vector.tensor_tensor(rc[:].rearrange("p (a l) -> p a l", a=n_p),
                                    rcj, rci, ALU.add)
            targ = wt("t0")
            nc.vector.tensor_tensor(targ[:], rc[:], rdr[:], ALU.mult)
            cnt = wt("t1")
            nc.scalar.activation(cnt[:], targ[:], AF.Sigmoid, scale=KCN, bias=b_negk[:])
            nc.vector.tensor_reduce(
                cn_t[:, acol:acol + n_p],
                cnt[:].rearrange("p (a l) -> p a l", a=n_p), AX.X, ALU.add)
            # ---- BJ damping factor D
            qq = bt("b0")
            r4ib = r43[:, acol:acol + n_p].unsqueeze(-1).to_broadcast([P, n_p, L])
            nc.vector.tensor_tensor(qq[:].rearrange("p (a l) -> p a l", a=n_p),
                                    sh[:].rearrange("p (a l) -> p a l", a=n_p),
                                    r4ib, ALU.mult)
            rrs = bt("b1")
            nc.scalar.activation(rrs[:], qq[:], AF.Sqrt, scale=A1 * A1)
            rr2 = bt("b2")
            nc.scalar.activation(rr2[:], rrs[:], AF.Square, bias=b_a2[:])
            t2_ = bt("b1")
            nc.scalar.activation(t2_[:], rr2[:], AF.Square)
            rr6 = bt("b3")
            nc.vector.tensor_tensor(rr6[:], t2_[:], rr2[:], ALU.mult)
            rr8 = bt("b1")
            nc.vector.tensor_tensor(rr8[:], rr6[:], rr2[:], ALU.mult)
            u = bt("b2")  # dr2^2
            nc.scalar.activation(u[:], s2[:], AF.Square, scale=RB)
            dr8 = bt("b4")
            nc.scalar.activation(dr8[:], u[:], AF.Square)
            dr6 = bt("b5")
            nc.vector.scalar_tensor_tensor(dr6[:], s2[:], RB, u[:], ALU.mult, ALU.mult)
            den6 = bt("b2")
            nc.vector.tensor_tensor(den6[:], dr6[:], rr6[:], ALU.add)
            den8 = bt("b3")
            nc.vector.tensor_tensor(den8[:], dr8[:], rr8[:], ALU.add)
            i6 = bt("b4")
            i8 = bt("b5")
            with nc.allow_low_precision(reason="bf16 damping chain, 2e-2 tol"):
                nc.vector.reciprocal(i6[:], den6[:])
                nc.vector.reciprocal(i8[:], den8[:])
            t8 = bt("b2")
            nc.vector.tensor_tensor(t8[:], qq[:], i8[:], ALU.mult)
            nc.vector.scalar_tensor_tensor(
                D_t[:, scol:scol + W], i6[:], S6 / S8, t8[:], ALU.mult, ALU.add)

          # ---- per-atom gaussian weights
          w5p = persist.tile([P, 5, ACOLS], F32, tag="w5p", name="w5p")
          attv = att[:].rearrange("p a f -> p f a")
          nc.vector.tensor_tensor(
              w5p[:], attv[:, 1:6, :],
              cn_t[:].unsqueeze(1).to_broadcast([P, 5, ACOLS]), ALU.subtract)
          sq = persist.tile([P, 5, ACOLS], F32, tag="sq", name="sq")
          nc.scalar.activation(sq[:], w5p[:], AF.Square)
          nc.scalar.activation(w5p[:], sq[:], AF.Exp, scale=-WF)
          wsum = persist.tile([P, ACOLS], F32, tag="wsum", name="wsum")
          nc.vector.tensor_tensor(wsum[:], w5p[:, 0, :], w5p[:, 1, :], ALU.add)
          nc.vector.tensor_tensor(wsum[:], wsum[:], w5p[:, 2, :], ALU.add)
          nc.vector.tensor_tensor(wsum[:], wsum[:], w5p[:, 3, :], ALU.add)
          nc.vector.tensor_tensor(wsum[:], wsum[:], w5p[:, 4, :], ALU.add)
          nc.vector.tensor_scalar_add(wsum[:], wsum[:], EPS32)
          winv = persist.tile([P, ACOLS], F32, tag="winv", name="winv")
          nc.vector.reciprocal(winv[:], wsum[:])
          wb = persist.tile([P, 5, ACOLS], BF16, tag="wb", name="wb")
          nc.vector.tensor_tensor(
              wb[:], w5p[:], winv[:].unsqueeze(1).to_broadcast([P, 5, ACOLS]),
              ALU.mult)

        nc.sync.dma_start(dout[:], D_t[:])
        nc.sync.dma_start(wout[:], wb[:].rearrange("p f a -> p (f a)"))
    nc.compile()
    return nc


# ---------------------------------------------------------------- launch 2
def _build_l2(NCHP, RND):
    nc = _new_nc()
    d2 = nc.declare_dram_parameter("d2", [P, NCHP], BF16, isOutput=False)
    wj5 = nc.declare_dram_parameter("wj5", [P, 5 * NCHP], BF16, isOutput=False)
    wi5 = nc.declare_dram_parameter("wi5", [P, 5 * NCHP], BF16, isOutput=False)
    mask = nc.declare_dram_parameter(
        "mask", [5 * GCH, RND * GPR * 5 * GCH], BF16, isOutput=False)
    eto = nc.declare_dram_parameter("etot", [1, 1], F32, isOutput=True)

    NT = NCHP // TCH
    M = 5 * GCH              # 40 psum rows
    RW = GPR * M             # 480 cols per round
    GPT = TCH // GCH         # groups per tile

    with ExitStack() as ctx:
        tc = ctx.enter_context(tile.TileContext(nc))
        persist = ctx.enter_context(tc.tile_pool(name="persist", bufs=1))
        spool = ctx.enter_context(tc.tile_pool(name="stream", bufs=2))
        apool = ctx.enter_context(tc.tile_pool(name="aw", bufs=2))
        bpool = ctx.enter_context(tc.tile_pool(name="workb", bufs=3))
        ppool = ctx.enter_context(tc.tile_pool(name="psum", bufs=4, space="PSUM"))
        cpool = ctx.enter_context(tc.tile_pool(name="psc", bufs=1, space="PSUM"))

        ones40 = persist.tile([M, 1], BF16)
        nc.vector.memset(ones40[:], 1.0)
        psC = cpool.tile([1, RW], F32)

        for rep in range(REPEAT):
          for t in range(NT):
            c0 = t * TCH
            dt_ = spool.tile([P, TCH], BF16, tag="dt")
            nc.sync.dma_start(dt_[:], d2[:, c0:c0 + TCH])
            # wj/wi are host-laid group-major: col = (g, r|s, c) for groups
            # of GCH chunks, so matmul operands are flat contiguous slices.
            wjt = spool.tile([P, 5 * TCH], BF16, tag="wjt")
            nc.sync.dma_start(wjt[:], wj5[:, c0 * 5:(c0 + TCH) * 5])
            wit = spool.tile([P, 5 * TCH], BF16, tag="wit")
            nc.sync.dma_start(wit[:], wi5[:, c0 * 5:(c0 + TCH) * 5])
            mt = spool.tile([M, RPT * RW], BF16, tag="mt")
            nc.sync.dma_start(mt[:], mask[:, t * RPT * RW:(t + 1) * RPT * RW])

            at = apool.tile([P, 5 * TCH], BF16, tag="at")
            dbc = (
                dt_[:].rearrange("p (g c) -> p g c", c=GCH)
                .unsqueeze(2).to_broadcast([P, GPT, 5, GCH])
            )
            nc.vector.tensor_tensor(
                at[:].rearrange("p (g r c) -> p g r c", g=GPT, r=5),
                wjt[:].rearrange("p (g r c) -> p g r c", g=GPT, r=5),
                dbc, ALU.mult)

            for rd in range(RPT):
                ps = ppool.tile([M, RW], F32, tag="ps")
                for g in range(GPR):
                    gl = rd * GPR + g
                    nc.tensor.matmul(
                        ps[:, g * M:(g + 1) * M],
                        at[:, gl * M:(gl + 1) * M],
                        wit[:, gl * M:(gl + 1) * M],
                        start=True, stop=True)
                bst = bpool.tile([M, RW], BF16, tag="bst")
                nc.scalar.copy(bst[:], ps[:])
                prod = bpool.tile([M, RW], BF16, tag="prod")
                nc.vector.tensor_tensor(
                    prod[:], bst[:], mt[:, rd * RW:(rd + 1) * RW], ALU.mult)
                glob = (rep * NT + t) * RPT + rd
                nc.tensor.matmul(
                    psC[:], ones40[:], prod[:],
                    start=(glob == 0), stop=(glob == REPEAT * NT * RPT - 1))

        g1 = persist.tile([1, RW], F32)
        nc.scalar.copy(g1[:], psC[:])
        er = persist.tile([1, 1], F32)
        nc.vector.tensor_reduce(
            er[:], g1[:].rearrange("a (b c) -> a b c", b=1), AX.X, ALU.add)
        nc.vector.tensor_scalar_mul(er[:], er[:], -S8 * HA / 2.0)
        nc.sync.dma_start(eto[:], er[:])
    nc.compile()
    return nc


# ---------------------------------------------------------------- host join
def _join(prep, r1_results):
    """Assemble L2 inputs from L1 outputs (gathers/permutations only)."""
    N, NCHP = prep["N"], prep["NCHP"]
    i, j = prep["i"], prep["j"]
    w_full = np.zeros((N, 5), ml_dtypes.bfloat16)
    d1 = []
    for k, c in enumerate(prep["cores"]):
        wk = np.asarray(r1_results[k]["w"]).reshape(P, 5, prep["ACOLS"])
        m = c["agrid"] >= 0
        w_full[c["agrid"][m]] = np.moveaxis(wk, 1, 2)[m]
        d1.append(np.asarray(r1_results[k]["D"]))
    # per-edge D via the L1 slot map
    Dfull = np.empty(prep["E"], ml_dtypes.bfloat16)
    ec, ep, ecc = prep["e_core"], prep["e_pp1"], prep["e_cc1"]
    for k in range(NCORES):
        m = ec == k
        Dfull[m] = d1[k][ep[m], ecc[m]]

    in2 = []
    for k in range(NCORES):
        c2 = prep["core2"][k]
        eo2, pp2, cc2 = c2["eo2"], c2["pp2"], c2["cc2"]
        d2 = np.zeros((P, NCHP), ml_dtypes.bfloat16)
        d2[pp2, cc2] = Dfull[eo2]
        # group-major (g, r|s, c) weight layout for 1-free-dim matmul APs
        wj = np.zeros((P, NCHP // GCH, 5, GCH), ml_dtypes.bfloat16)
        wj[pp2, cc2 // GCH, :, cc2 % GCH] = w_full[j[eo2]]
        wi = np.zeros((P, NCHP // GCH, 5, GCH), ml_dtypes.bfloat16)
        wi[pp2, cc2 // GCH, :, cc2 % GCH] = w_full[i[eo2]]
        in2.append({
            "d2": d2,
            "wj5": wj.reshape(P, -1),
            "wi5": wi.reshape(P, -1),
            "mask": c2["mask"],
        })
    return in2


def kernel(dr_vec, ref_cn_table, ref_c6_table, r4r2_table, rcov_table, numbers, idx):
    # smooth_cutoff(dr, 20, 25) and (55, 60) are identically 1 for this data
    assert np.sqrt((dr_vec.astype(np.float64) ** 2).sum(-1)).max() / BOHR < 19.0
    prep = _prep(dr_vec, ref_cn_table, ref_c6_table, r4r2_table, rcov_table,
                 numbers, idx)
    key = (tuple(prep["pieces"]), prep["COLS"], prep["ACOLS"], prep["NCHP"])
    if key not in _cache:
        _cache[key] = (
            _build_l1(prep["pieces"], prep["COLS"], prep["ACOLS"]),
            _build_l2(prep["NCHP"], prep["RND"]),
        )
    nc1, nc2 = _cache[key]

    in1 = [
        {"s1": c["s1"].reshape(P, -1), "s1h": c["s1h"], "at2": c["at2"].reshape(P, -1)}
        for c in prep["cores"]
    ]
    r1 = run_bass_kernel_spmd(nc1, in1, list(range(NCORES)), trace=TRACE)
    in2 = _join(prep, r1.results)
    r2 = run_bass_kernel_spmd(nc2, in2, list(range(NCORES)), trace=TRACE)
    parts = [np.asarray(r2.results[k]["etot"]).reshape(()) for k in range(NCORES)]
    return np.float32(np.sum(np.stack(parts)))
